# revision 41
# baseline (speedup 1.0000x reference)
"""Bidirectional LSTM layer (T=256, B=64, I=H=512) on 8 TRN2 NeuronCores.

Sharding (all modes): core = direction(2) x batch-shard(4), 16 samples per
core. The backward direction is handled by feeding time-reversed x to cores
4-7 and re-reversing on the host, so all 8 cores run one identical SPMD
graph. All shard/unshard work is numpy inside kernel(). Numerics: bf16
weights/activations into the PE, f32 PSUM accumulation and f32 cell state
(validated 1.1e-2 absmax rel err vs the f32 reference).

MODE="sel" (default, instruction-minimal -- execution on this rig is priced
per instruction, not per FLOP): gates live in [batch, gate] orientation.
  - xg(+biases) for 8 timesteps at a time is one [128(=8 steps x 16 batch),
    2048] accumulation group (x-chunks stationary, WiT streamed; biases
    injected by a K=1 ones-row outer-product matmul).
  - per step: a selector-matrix matmul pulls that step's 16 xg rows into the
    gate PSUM [16, 2048], then 16 matmuls (h-chunks stationary, WhT streamed)
    add the recurrent term; sigmoid/tanh + cell update on [16, 512*k] tiles;
    one dma_start_transpose returns h to the [128, batch] stationary layout.

MODE="local": orientation-B variant (Wh tiles stationary, hT streamed, xg
pre-GEMM resident in SBUF, identity-matmul xg injection). ~2.3x more
instructions; better under cost models that charge per streamed column.

MODE="tp4": gate-tensor-parallel across 4-core groups with per-step h
exchange via remote_dma_broadcast -- unusable on this rig (remote DMA
crashes the runtime) but kept for environments where it works.
"""

import sys

for p in ("/opt/trn_rl_repo",):
    if p not in sys.path:
        sys.path.insert(0, p)

import numpy as np
import ml_dtypes

import concourse.bass as bass
import concourse.tile as tile
import concourse.mybir as mybir
from concourse import bacc, bass_utils

BF16 = ml_dtypes.bfloat16
F32 = np.float32

T, B, I, H = 256, 64, 512, 512
GATE_ORDER = [0, 1, 3, 2]  # pytorch i,f,g,o -> our row-tile order i,f,o,g

MODE = "sel"  # "sel" (instruction-minimal), "local", or "tp4" (needs remote_dma)
# Fused variant (recurrence accumulating straight into the GEMM PSUM block)
# passes CoreSim but fails walrus BIR verification: engine accesses need
# 32-aligned partition bases, and odd steps would read gates at offset 16.
# The selector matmul in the default path is the legal 16-row extraction.
SEL_FUSED = False
# Packing the 9 input arrays into one bf16 blob measured as a wash (A/B within
# +-0.15s noise): the transfer floor is bandwidth-bound, not per-array.
SEL_PACKED = False

TRACE = False  # set by test harness to capture neuron-profile timing
LAST_RESULT = None  # BassKernelResults of the most recent run

_GRAPH_CACHE = {}


def _cfg(mode):
    if mode == "local":
        # Bs batch per core, RT gate row-tiles per core (16 = all 2048 rows)
        return dict(Bs=16, RT=16, use_remote=False)
    elif mode == "tp4":
        return dict(Bs=64, RT=4, use_remote=True)
    elif mode == "sel":
        return dict(Bs=16, RT=16, use_remote=False)
    raise ValueError(mode)


def build_graph_sel_fused(Tn=T):
    """sel-mode refinement: no selector matmuls, no xg staging.

    The recurrence W-matmuls accumulate directly into the 8-step GEMM PSUM
    block. PE matmul outputs must start at a 32-aligned partition, so the
    stationary h operand is padded to 32 columns with a zero half selected by
    step parity: the real result lands on the step's 16 rows, zeros
    accumulate (+0) onto the neighbor step's rows. Per step: 16 matmul pairs
    + activations + one DMA transpose.
    """
    Bs, G = 16, 2048
    dt = mybir.dt
    NBLK = Tn // 8

    nc = bacc.Bacc("TRN2", target_bir_lowering=False, debug=False, num_devices=8)

    xT_d = nc.dram_tensor("xT", [4, 128, Tn * Bs], dt.bfloat16, kind="ExternalInput").ap()
    wiT_d = nc.dram_tensor("wiT", [4, 128, G], dt.bfloat16, kind="ExternalInput").ap()
    whT_d = nc.dram_tensor("whT", [4, 128, G], dt.bfloat16, kind="ExternalInput").ap()
    bias_d = nc.dram_tensor("bias", [1, G], dt.bfloat16, kind="ExternalInput").ap()
    ones_d = nc.dram_tensor("ones", [1, 128], dt.bfloat16, kind="ExternalInput").ap()
    sel_d = nc.dram_tensor("sel", [8, 128, Bs], dt.bfloat16, kind="ExternalInput").ap()
    h0T_d = nc.dram_tensor("h0T", [128, 4, Bs], dt.bfloat16, kind="ExternalInput").ap()
    c0_d = nc.dram_tensor("c0", [Bs, 512], dt.float32, kind="ExternalInput").ap()
    id16_d = nc.dram_tensor("id16", [16, 16], dt.bfloat16, kind="ExternalInput").ap()

    ysT_d = nc.dram_tensor("ysT", [Bs, Tn, 512], dt.bfloat16, kind="ExternalOutput").ap()
    hfin_d = nc.dram_tensor("hfin", [Bs, 512], dt.float32, kind="ExternalOutput").ap()
    cfin_d = nc.dram_tensor("cfin", [Bs, 512], dt.float32, kind="ExternalOutput").ap()

    AF = mybir.ActivationFunctionType

    with tile.TileContext(nc) as tc:
        with (
            tc.tile_pool(name="persist", bufs=1) as persist,
            tc.tile_pool(name="gps", bufs=2, space="PSUM") as gpsum,
            tc.tile_pool(name="ew", bufs=2) as ew,
            tc.tile_pool(name="hsb", bufs=2) as hsbp,
        ):
            xT = persist.tile([128, 4, Tn * Bs], dt.bfloat16, tag="xT")
            wiT = persist.tile([128, 4, G], dt.bfloat16, tag="wiT")
            whT = persist.tile([128, 4, G], dt.bfloat16, tag="whT")
            bias = persist.tile([1, G], dt.bfloat16, tag="bias")
            ones = persist.tile([1, 128], dt.bfloat16, tag="ones")
            # ring slot r holds h in column half r, zeros in the other half
            hT = persist.tile([128, 2, 4, 2 * Bs], dt.bfloat16, tag="hT")
            c_sb = persist.tile([Bs, 512], dt.float32, tag="c_sb")

            for k in range(4):
                nc.sync.dma_start(xT[:, k, :], xT_d[k])
                nc.sync.dma_start(wiT[:, k, :], wiT_d[k])
                nc.sync.dma_start(whT[:, k, :], whT_d[k])
            nc.sync.dma_start(bias[:], bias_d[:])
            nc.sync.dma_start(ones[:], ones_d[:])
            nc.gpsimd.memset(hT[:], 0.0)
            nc.sync.dma_start(hT[:, 0, :, 0:Bs], h0T_d[:])
            nc.sync.dma_start(c_sb[:], c0_d[:])

            ps_tiles = {}

            def emit_gemm_block(blk):
                ps = gpsum.tile([128, G], dt.float32, tag="gps", name=f"gps{blk}")
                for n in range(4):
                    nsl = slice(n * 512, (n + 1) * 512)
                    nc.tensor.matmul(
                        ps[:, nsl], ones[:], bias[:, nsl],
                        start=True, stop=False, skip_group_check=True,
                    )
                for k in range(4):
                    xsl = xT[:, k, blk * 128 : (blk + 1) * 128]
                    for n in range(4):
                        nsl = slice(n * 512, (n + 1) * 512)
                        nc.tensor.matmul(
                            ps[:, nsl], xsl, wiT[:, k, nsl],
                            start=False, stop=False, skip_group_check=True,
                        )
                ps_tiles[blk] = ps

            emit_gemm_block(0)

            for t in range(Tn):
                blk, ph = t // 8, t % 8
                cur, nxt = t % 2, (t + 1) % 2
                if ph == 4 and blk + 1 < NBLK:
                    emit_gemm_block(blk + 1)
                ps = ps_tiles[blk]
                off = 32 * (ph // 2)
                rsl = slice(16 * ph, 16 * ph + 16)

                for k in range(4):
                    hsl = hT[:, cur, k, :]
                    for n in range(4):
                        nsl = slice(n * 512, (n + 1) * 512)
                        nc.tensor.matmul(
                            ps[off : off + 32, nsl], hsl, whT[:, k, nsl],
                            start=False, stop=(ph == 7 and k == 3),
                            skip_group_check=True, tile_position=(0, off),
                        )

                # gate columns: [i | f | o | g]; this step's rows = rsl
                sig = ew.tile([Bs, 1536], dt.float32, tag="sig")
                tng = ew.tile([Bs, 512], dt.float32, tag="tng")
                nc.scalar.activation(sig[:], ps[rsl, 0:1536], AF.Sigmoid)
                nc.scalar.activation(tng[:], ps[rsl, 1536:2048], AF.Tanh)
                ig = ew.tile([Bs, 512], dt.float32, tag="ig")
                fc = ew.tile([Bs, 512], dt.float32, tag="fc")
                nc.vector.tensor_mul(ig[:], sig[:, 0:512], tng[:])
                nc.vector.tensor_mul(fc[:], sig[:, 512:1024], c_sb[:])
                nc.vector.tensor_add(c_sb[:], ig[:], fc[:])
                tcn = ew.tile([Bs, 512], dt.float32, tag="tc")
                nc.scalar.activation(tcn[:], c_sb[:], AF.Tanh)

                if ph == 0:
                    h_sb = hsbp.tile([Bs, 8, 512], dt.bfloat16, tag="h_sb", name=f"hsb{blk}")
                nc.vector.tensor_mul(h_sb[:, ph, :], sig[:, 1024:1536], tcn[:])

                if t < Tn - 1:
                    nc.sync.dma_start_transpose(
                        hT[:, nxt, :, 16 * nxt : 16 * nxt + 16], h_sb[:, ph, :]
                    )

                if ph == 7:
                    nc.sync.dma_start(ysT_d[:, t - 7 : t + 1, :], h_sb[:, :, :])
                if t == Tn - 1:
                    hf = ew.tile([Bs, 512], dt.float32, tag="hf")
                    nc.vector.tensor_mul(hf[:], sig[:, 1024:1536], tcn[:])
                    nc.sync.dma_start(hfin_d[:], hf[:])
                    nc.sync.dma_start(cfin_d[:], c_sb[:])

    nc.compile()
    return nc


def _sel_blob_layout(Tn):
    """(name, elems, shape) of each bf16 segment in the packed input blob.
    c0 (f32) rides as raw bytes viewed as 2x bf16 elements."""
    Bs, G = 16, 2048
    TBs = Tn * Bs
    segs = [
        ("xT", 4 * 128 * TBs, (4, 128, TBs)),
        ("wiT", 4 * 128 * G, (4, 128, G)),
        ("whT", 4 * 128 * G, (4, 128, G)),
        ("bias", G, (1, G)),
        ("ones", 128, (1, 128)),
        ("sel", 8 * 128 * Bs, (8, 128, Bs)),
        ("h0T", 128 * 4 * Bs, (128, 4, Bs)),
        ("c0u", Bs * 1024, (Bs, 1024)),
    ]
    offs, o = {}, 0
    for name, n, shape in segs:
        offs[name] = (o, n, shape)
        o += n
    return offs, o


def build_graph_sel_packed(Tn=T):
    """sel graph with all inputs packed into one bf16 blob (the axon tunnel
    charges ~20ms per array per call, so 9 arrays -> 1 saves ~0.15-0.2s)."""
    Bs, G = 16, 2048
    dt = mybir.dt
    NBLK = Tn // 8
    offs, total = _sel_blob_layout(Tn)

    nc = bacc.Bacc("TRN2", target_bir_lowering=False, debug=False, num_devices=8)
    blob_d = nc.dram_tensor("blob", [total], dt.bfloat16, kind="ExternalInput").ap()
    ysT_d = nc.dram_tensor("ysT", [Bs, Tn, 512], dt.bfloat16, kind="ExternalOutput").ap()
    hfin_d = nc.dram_tensor("hfin", [Bs, 512], dt.float32, kind="ExternalOutput").ap()
    cfin_d = nc.dram_tensor("cfin", [Bs, 512], dt.float32, kind="ExternalOutput").ap()

    def seg(name, idx=None):
        o, n, shape = offs[name]
        if idx is not None:  # slice leading dim
            per = n // shape[0]
            sl = blob_d[o + idx * per : o + (idx + 1) * per]
            rest = shape[1:]
            if len(rest) == 1:
                return sl.rearrange("(a) -> a")
            return sl.rearrange("(p f) -> p f", p=rest[0])
        sl = blob_d[o : o + n]
        if len(shape) == 2:
            return sl.rearrange("(p f) -> p f", p=shape[0])
        return sl.rearrange("(p k f) -> p k f", p=shape[0], k=shape[1])

    AF = mybir.ActivationFunctionType
    with tile.TileContext(nc) as tc:
        with (
            tc.tile_pool(name="persist", bufs=1) as persist,
            tc.tile_pool(name="xgp", bufs=2) as xgp,
            tc.tile_pool(name="gps", bufs=1, space="PSUM") as gpsum,
            tc.tile_pool(name="rps", bufs=1, space="PSUM") as rpsum,
            tc.tile_pool(name="ew", bufs=4) as ew,
            tc.tile_pool(name="hsb", bufs=2) as hsbp,
        ):
            xT = persist.tile([128, 4, Tn * Bs], dt.bfloat16, tag="xT")
            wiT = persist.tile([128, 4, G], dt.bfloat16, tag="wiT")
            whT = persist.tile([128, 4, G], dt.bfloat16, tag="whT")
            bias = persist.tile([1, G], dt.bfloat16, tag="bias")
            ones = persist.tile([1, 128], dt.bfloat16, tag="ones")
            sel = persist.tile([128, 8, Bs], dt.bfloat16, tag="sel")
            hT = persist.tile([128, 2, 4, Bs], dt.bfloat16, tag="hT")
            c_sb = persist.tile([Bs, 512], dt.float32, tag="c_sb")

            for k in range(4):
                nc.sync.dma_start(xT[:, k, :], seg("xT", k))
                nc.sync.dma_start(wiT[:, k, :], seg("wiT", k))
                nc.sync.dma_start(whT[:, k, :], seg("whT", k))
            nc.sync.dma_start(bias[:], seg("bias"))
            nc.sync.dma_start(ones[:], seg("ones"))
            for s in range(8):
                nc.sync.dma_start(sel[:, s, :], seg("sel", s))
            nc.sync.dma_start(hT[:, 0, :, :], seg("h0T"))
            nc.sync.dma_start(c_sb[:].bitcast(dt.bfloat16), seg("c0u"))

            xg_tiles = {}

            def emit_gemm_block(blk):
                ps = gpsum.tile([128, G], dt.float32, tag="gps", name=f"gps{blk}")
                for n in range(4):
                    nsl = slice(n * 512, (n + 1) * 512)
                    nc.tensor.matmul(
                        ps[:, nsl], ones[:], bias[:, nsl],
                        start=True, stop=False, skip_group_check=True,
                    )
                for k in range(4):
                    xsl = xT[:, k, blk * 128 : (blk + 1) * 128]
                    for n in range(4):
                        nsl = slice(n * 512, (n + 1) * 512)
                        nc.tensor.matmul(
                            ps[:, nsl], xsl, wiT[:, k, nsl],
                            start=False, stop=(k == 3), skip_group_check=True,
                        )
                xg = xgp.tile([128, G], dt.bfloat16, tag="xg", name=f"xg{blk}")
                nc.scalar.activation(xg[:, 0:1024], ps[:, 0:1024], AF.Copy)
                nc.vector.tensor_copy(xg[:, 1024:2048], ps[:, 1024:2048])
                xg_tiles[blk] = xg

            emit_gemm_block(0)

            for t in range(Tn):
                blk, ph = t // 8, t % 8
                cur, nxt = t % 2, (t + 1) % 2
                if ph == 4 and blk + 1 < NBLK:
                    emit_gemm_block(blk + 1)
                xg = xg_tiles[blk]

                ps = rpsum.tile([Bs, G], dt.float32, tag="rps", name=f"rps{t}")
                for n in range(4):
                    nsl = slice(n * 512, (n + 1) * 512)
                    nc.tensor.matmul(
                        ps[:, nsl], sel[:, ph, :], xg[:, nsl],
                        start=True, stop=False, skip_group_check=True,
                    )
                for k in range(4):
                    hsl = hT[:, cur, k, :]
                    for n in range(4):
                        nsl = slice(n * 512, (n + 1) * 512)
                        nc.tensor.matmul(
                            ps[:, nsl], hsl, whT[:, k, nsl],
                            start=False, stop=(k == 3), skip_group_check=True,
                        )

                sig = ew.tile([Bs, 1536], dt.float32, tag="sig")
                tng = ew.tile([Bs, 512], dt.float32, tag="tng")
                nc.scalar.activation(sig[:], ps[:, 0:1536], AF.Sigmoid)
                nc.scalar.activation(tng[:], ps[:, 1536:2048], AF.Tanh)
                ig = ew.tile([Bs, 512], dt.float32, tag="ig")
                fc = ew.tile([Bs, 512], dt.float32, tag="fc")
                nc.vector.tensor_mul(ig[:], sig[:, 0:512], tng[:])
                nc.vector.tensor_mul(fc[:], sig[:, 512:1024], c_sb[:])
                nc.vector.tensor_add(c_sb[:], ig[:], fc[:])
                tcn = ew.tile([Bs, 512], dt.float32, tag="tc")
                nc.scalar.activation(tcn[:], c_sb[:], AF.Tanh)

                if ph == 0:
                    h_sb = hsbp.tile([Bs, 8, 512], dt.bfloat16, tag="h_sb", name=f"hsb{blk}")
                nc.vector.tensor_mul(h_sb[:, ph, :], sig[:, 1024:1536], tcn[:])

                if t < Tn - 1:
                    nc.sync.dma_start_transpose(hT[:, nxt, :, :], h_sb[:, ph, :])

                if ph == 7:
                    nc.sync.dma_start(ysT_d[:, t - 7 : t + 1, :], h_sb[:, :, :])
                if t == Tn - 1:
                    hf = ew.tile([Bs, 512], dt.float32, tag="hf")
                    nc.vector.tensor_mul(hf[:], sig[:, 1024:1536], tcn[:])
                    nc.sync.dma_start(hfin_d[:], hf[:])
                    nc.sync.dma_start(cfin_d[:], c_sb[:])

    nc.compile()
    return nc


def build_graph_sel(Tn=T, use_dma_transpose=True):
    """Instruction-minimal variant: gates in [batch, gate] orientation.

    xg for 8 steps at a time is one [128, 2048] GEMM block (partitions =
    8 steps x 16 batch, bias via a K=1 ones-row matmul); each step's xg rows
    are pulled into the gate PSUM with a selector-matrix matmul, followed by
    h-chunk-stationary matmuls streaming WhT. h goes back to the [128, b]
    stationary layout with one DMA transpose per step.
    """
    Bs, G = 16, 2048
    dt = mybir.dt
    NBLK = Tn // 8

    nc = bacc.Bacc("TRN2", target_bir_lowering=False, debug=False, num_devices=8)

    xT_d = nc.dram_tensor("xT", [4, 128, Tn * Bs], dt.bfloat16, kind="ExternalInput").ap()
    wiT_d = nc.dram_tensor("wiT", [4, 128, G], dt.bfloat16, kind="ExternalInput").ap()
    whT_d = nc.dram_tensor("whT", [4, 128, G], dt.bfloat16, kind="ExternalInput").ap()
    bias_d = nc.dram_tensor("bias", [1, G], dt.bfloat16, kind="ExternalInput").ap()
    ones_d = nc.dram_tensor("ones", [1, 128], dt.bfloat16, kind="ExternalInput").ap()
    sel_d = nc.dram_tensor("sel", [8, 128, Bs], dt.bfloat16, kind="ExternalInput").ap()
    h0T_d = nc.dram_tensor("h0T", [128, 4, Bs], dt.bfloat16, kind="ExternalInput").ap()
    c0_d = nc.dram_tensor("c0", [Bs, 512], dt.float32, kind="ExternalInput").ap()
    id16_d = nc.dram_tensor("id16", [16, 16], dt.bfloat16, kind="ExternalInput").ap()

    ysT_d = nc.dram_tensor("ysT", [Bs, Tn, 512], dt.bfloat16, kind="ExternalOutput").ap()
    hfin_d = nc.dram_tensor("hfin", [Bs, 512], dt.float32, kind="ExternalOutput").ap()
    cfin_d = nc.dram_tensor("cfin", [Bs, 512], dt.float32, kind="ExternalOutput").ap()

    AF = mybir.ActivationFunctionType

    with tile.TileContext(nc) as tc:
        with (
            tc.tile_pool(name="persist", bufs=1) as persist,
            tc.tile_pool(name="xgp", bufs=2) as xgp,
            tc.tile_pool(name="gps", bufs=1, space="PSUM") as gpsum,
            tc.tile_pool(name="rps", bufs=1, space="PSUM") as rpsum,
            tc.tile_pool(name="ew", bufs=4) as ew,
            tc.tile_pool(name="hsb", bufs=2) as hsbp,
        ):
            xT = persist.tile([128, 4, Tn * Bs], dt.bfloat16, tag="xT")
            wiT = persist.tile([128, 4, G], dt.bfloat16, tag="wiT")
            whT = persist.tile([128, 4, G], dt.bfloat16, tag="whT")
            bias = persist.tile([1, G], dt.bfloat16, tag="bias")
            ones = persist.tile([1, 128], dt.bfloat16, tag="ones")
            sel = persist.tile([128, 8, Bs], dt.bfloat16, tag="sel")
            id16 = persist.tile([16, 16], dt.bfloat16, tag="id16")
            hT = persist.tile([128, 2, 4, Bs], dt.bfloat16, tag="hT")
            c_sb = persist.tile([Bs, 512], dt.float32, tag="c_sb")

            for k in range(4):
                nc.sync.dma_start(xT[:, k, :], xT_d[k])
                nc.sync.dma_start(wiT[:, k, :], wiT_d[k])
                nc.sync.dma_start(whT[:, k, :], whT_d[k])
            nc.sync.dma_start(bias[:], bias_d[:])
            nc.sync.dma_start(ones[:], ones_d[:])
            for s in range(8):
                nc.sync.dma_start(sel[:, s, :], sel_d[s])
            nc.sync.dma_start(id16[:], id16_d[:])
            nc.sync.dma_start(hT[:, 0, :, :], h0T_d[:])
            nc.sync.dma_start(c_sb[:], c0_d[:])

            xg_tiles = {}

            def emit_gemm_block(blk):
                # xg for steps 8*blk..8*blk+7: [128 (8 steps x 16 batch), 2048]
                ps = gpsum.tile([128, G], dt.float32, tag="gps", name=f"gps{blk}")
                for n in range(4):
                    nsl = slice(n * 512, (n + 1) * 512)
                    nc.tensor.matmul(
                        ps[:, nsl], ones[:], bias[:, nsl],
                        start=True, stop=False, skip_group_check=True,
                    )
                for k in range(4):
                    xsl = xT[:, k, blk * 128 : (blk + 1) * 128]
                    for n in range(4):
                        nsl = slice(n * 512, (n + 1) * 512)
                        nc.tensor.matmul(
                            ps[:, nsl], xsl, wiT[:, k, nsl],
                            start=False, stop=(k == 3), skip_group_check=True,
                        )
                xg = xgp.tile([128, G], dt.bfloat16, tag="xg", name=f"xg{blk}")
                nc.scalar.activation(xg[:, 0:1024], ps[:, 0:1024], AF.Copy)
                nc.vector.tensor_copy(xg[:, 1024:2048], ps[:, 1024:2048])
                xg_tiles[blk] = xg

            emit_gemm_block(0)

            for t in range(Tn):
                blk, ph = t // 8, t % 8
                cur, nxt = t % 2, (t + 1) % 2
                if ph == 4 and blk + 1 < NBLK:
                    emit_gemm_block(blk + 1)
                xg = xg_tiles[blk]

                ps = rpsum.tile([Bs, G], dt.float32, tag="rps", name=f"rps{t}")
                for n in range(4):
                    nsl = slice(n * 512, (n + 1) * 512)
                    nc.tensor.matmul(
                        ps[:, nsl], sel[:, ph, :], xg[:, nsl],
                        start=True, stop=False, skip_group_check=True,
                    )
                for k in range(4):
                    hsl = hT[:, cur, k, :]
                    for n in range(4):
                        nsl = slice(n * 512, (n + 1) * 512)
                        nc.tensor.matmul(
                            ps[:, nsl], hsl, whT[:, k, nsl],
                            start=False, stop=(k == 3), skip_group_check=True,
                        )

                # gate columns: [i | f | o | g] along the 2048
                sig = ew.tile([Bs, 1536], dt.float32, tag="sig")
                tng = ew.tile([Bs, 512], dt.float32, tag="tng")
                nc.scalar.activation(sig[:], ps[:, 0:1536], AF.Sigmoid)
                nc.scalar.activation(tng[:], ps[:, 1536:2048], AF.Tanh)
                ig = ew.tile([Bs, 512], dt.float32, tag="ig")
                fc = ew.tile([Bs, 512], dt.float32, tag="fc")
                nc.vector.tensor_mul(ig[:], sig[:, 0:512], tng[:])
                nc.vector.tensor_mul(fc[:], sig[:, 512:1024], c_sb[:])
                nc.vector.tensor_add(c_sb[:], ig[:], fc[:])
                tcn = ew.tile([Bs, 512], dt.float32, tag="tc")
                nc.scalar.activation(tcn[:], c_sb[:], AF.Tanh)

                if ph == 0:
                    h_sb = hsbp.tile([Bs, 8, 512], dt.bfloat16, tag="h_sb", name=f"hsb{blk}")
                    xg_tiles[blk - 1] = None  # allow pool reuse
                nc.vector.tensor_mul(h_sb[:, ph, :], sig[:, 1024:1536], tcn[:])

                if t < Tn - 1:
                    if use_dma_transpose:
                        nc.sync.dma_start_transpose(hT[:, nxt, :, :], h_sb[:, ph, :])
                    else:
                        for k in range(4):
                            tp = rpsum.tile([128, Bs], dt.bfloat16, tag="tp", name=f"tp{t}_{k}")
                            nc.tensor.transpose(
                                tp[:], h_sb[:, ph, k * 128 : (k + 1) * 128], id16[:]
                            )
                            nc.vector.tensor_copy(hT[:, nxt, k, :], tp[:])

                if ph == 7:
                    nc.sync.dma_start(ysT_d[:, t - 7 : t + 1, :], h_sb[:, :, :])
                if t == Tn - 1:
                    hf = ew.tile([Bs, 512], dt.float32, tag="hf")
                    nc.vector.tensor_mul(hf[:], sig[:, 1024:1536], tcn[:])
                    nc.sync.dma_start(hfin_d[:], hf[:])
                    nc.sync.dma_start(cfin_d[:], c_sb[:])

    nc.compile()
    return nc


def build_graph(mode, Tn=T, reps=1):
    # reps>1 re-runs the recurrence compute without I/O (timing calibration
    # only -- outputs stay those of rep 0).
    if mode == "sel":
        assert reps == 1
        if SEL_FUSED:
            return build_graph_sel_fused(Tn)
        return build_graph_sel_packed(Tn) if SEL_PACKED else build_graph_sel(Tn)
    cfg = _cfg(mode)
    Bs, RT, use_remote = cfg["Bs"], cfg["RT"], cfg["use_remote"]
    RT4 = RT // 4
    TBs = Tn * Bs
    NCH = 8  # xg chunks along time
    CH = TBs // NCH  # columns per xg chunk
    NPW = min(512, CH)  # N-pass width
    NB = CH // NPW  # N passes per chunk
    dt = mybir.dt

    nc = bacc.Bacc("TRN2", target_bir_lowering=False, debug=False, num_devices=8)

    xT_d = nc.dram_tensor("xT", [4, 128, TBs], dt.bfloat16, kind="ExternalInput").ap()
    wiT_d = nc.dram_tensor("wiT", [4, 128, RT * 128], dt.bfloat16, kind="ExternalInput").ap()
    whT_d = nc.dram_tensor("whT", [4, 128, RT * 128], dt.bfloat16, kind="ExternalInput").ap()
    bias_d = nc.dram_tensor("bias", [128, RT], dt.float32, kind="ExternalInput").ap()
    h0T_d = nc.dram_tensor("h0T", [128, 4, Bs], dt.bfloat16, kind="ExternalInput").ap()
    c0T_d = nc.dram_tensor("c0T", [128, RT4, Bs], dt.float32, kind="ExternalInput").ap()
    id_d = nc.dram_tensor("ident", [128, 128], dt.bfloat16, kind="ExternalInput").ap()

    ysT_d = nc.dram_tensor("ysT", [Tn, 128, RT4 * Bs], dt.bfloat16, kind="ExternalOutput").ap()
    hfin_d = nc.dram_tensor("hfin", [128, RT4 * Bs], dt.float32, kind="ExternalOutput").ap()
    cfin_d = nc.dram_tensor("cfin", [128, RT4 * Bs], dt.float32, kind="ExternalOutput").ap()

    if use_remote:
        h_sem = nc.alloc_semaphore("h_sem")
        send_sem = nc.alloc_semaphore("send_sem")

    AF = mybir.ActivationFunctionType

    with tile.TileContext(nc) as tc:
        with (
            tc.tile_pool(name="persist", bufs=1) as persist,
            tc.tile_pool(name="xc", bufs=3) as xcp,
            tc.tile_pool(name="gpsum", bufs=3, space="PSUM") as gpsum,
            tc.tile_pool(name="spsum", bufs=2, space="PSUM") as spsum,
            tc.tile_pool(name="ew", bufs=2) as ew,
        ):
            wiT = persist.tile([128, 4, RT * 128], dt.bfloat16, tag="wiT")
            whT = persist.tile([128, 4, RT * 128], dt.bfloat16, tag="whT")
            bias = persist.tile([128, RT], dt.float32, tag="bias")
            ident = persist.tile([128, 128], dt.bfloat16, tag="ident")
            hT = persist.tile([128, 2, 4, Bs], dt.bfloat16, tag="hT")
            c_sb = persist.tile([128, RT4, Bs], dt.float32, tag="c_sb")
            xg = [
                persist.tile([128, RT, CH], dt.bfloat16, tag=f"xg{ch}", name=f"xg{ch}")
                for ch in range(NCH)
            ]

            for k in range(4):
                nc.sync.dma_start(wiT[:, k, :], wiT_d[k])
                nc.sync.dma_start(whT[:, k, :], whT_d[k])
            nc.sync.dma_start(bias[:], bias_d[:])
            nc.sync.dma_start(ident[:], id_d[:])
            nc.sync.dma_start(hT[:, 0, :, :], h0T_d[:])
            nc.sync.dma_start(c_sb[:], c0T_d[:])

            # ---- xg = x @ Wi^T + bias, bf16, resident in SBUF ----
            # One "work item" = (ch, nb, rt): 4 matmuls + 1 bias epilogue. The
            # xc chunk DMA is issued at the first item of each (ch, nb).
            xc_tiles = {}

            def emit_gemm_item(ch, nb, rt):
                off = ch * CH + nb * NPW
                if rt == 0:
                    xc = xcp.tile([128, 4, NPW], dt.bfloat16, tag="xc", name=f"xc{ch}_{nb}")
                    for k in range(4):
                        nc.sync.dma_start(xc[:, k, :], xT_d[k, :, off : off + NPW])
                    xc_tiles[(ch, nb)] = xc
                xc = xc_tiles[(ch, nb)]
                ps = gpsum.tile([128, NPW], dt.float32, tag="gps", name=f"gps{ch}_{nb}_{rt}")
                for k in range(4):
                    nc.tensor.matmul(
                        ps[:],
                        wiT[:, k, rt * 128 : (rt + 1) * 128],
                        xc[:, k, :],
                        start=(k == 0),
                        stop=(k == 3),
                    )
                dst = xg[ch][:, rt, nb * NPW : (nb + 1) * NPW]
                if rt % 2 == 0:
                    nc.scalar.activation(
                        dst, ps[:], AF.Identity, bias=bias[:, rt : rt + 1]
                    )
                else:
                    nc.vector.tensor_scalar_add(dst, ps[:], bias[:, rt : rt + 1])

            # chunks 0..PRE-1 upfront; chunks PRE.. are interleaved into the
            # recurrence steps of chunk c-PRE (PE consumes them in its idle
            # windows while the elementwise chain runs).
            PRE = 1
            for ch in range(PRE):
                for nb in range(NB):
                    for rt in range(RT):
                        emit_gemm_item(ch, nb, rt)

            TCH_steps = Tn // NCH
            items_per_step = -(-(NB * RT) // TCH_steps)  # ceil

            def emit_interleaved_gemm(t):
                ch = t // TCH_steps + PRE
                if ch >= NCH:
                    return
                pos = t % TCH_steps
                for it in range(pos * items_per_step, min((pos + 1) * items_per_step, NB * RT)):
                    emit_gemm_item(ch, it // RT, it % RT)

            # ---- recurrence ----
            for rep in range(reps):
              for t in range(Tn):
                cur, nxt = t % 2, (t + 1) % 2
                ch, tloc = t // TCH_steps, t % TCH_steps
                xg_t = xg[ch]
                ps = spsum.tile([128, RT, Bs], dt.float32, tag="sps")

                # xg injection via identity matmuls (N = RT*Bs/n_idmm <= 512)
                n_idmm = max(1, (RT * Bs) // 512)
                rt_per = RT // n_idmm
                for q in range(n_idmm):
                    nc.tensor.matmul(
                        ps[:, q * rt_per : (q + 1) * rt_per, :],
                        ident[:],
                        xg_t[:, q * rt_per : (q + 1) * rt_per, tloc * Bs : (tloc + 1) * Bs],
                        start=(q == 0),
                        stop=False,
                        skip_group_check=True,
                    )

                def wh_mms():
                    for rt in range(RT):
                        for k in range(4):
                            nc.tensor.matmul(
                                ps[:, rt, :],
                                whT[:, k, rt * 128 : (rt + 1) * 128],
                                hT[:, cur, k, :],
                                start=False,
                                stop=(rt == RT - 1 and k == 3),
                                skip_group_check=True,
                            )

                if use_remote and t >= 1:
                    with tc.tile_critical():
                        nc.tensor.wait_ge(h_sem, 6 * t)
                        wh_mms()
                else:
                    wh_mms()

                # elementwise: row-tiles [0:RT4]=i [RT4:2RT4]=f [2RT4:3RT4]=o [3RT4:RT]=g
                sig = ew.tile([128, 3 * RT4, Bs], dt.float32, tag="sig")
                tng = ew.tile([128, RT4, Bs], dt.float32, tag="tng")
                nc.scalar.activation(sig[:], ps[:, 0 : 3 * RT4, :], AF.Sigmoid)
                nc.scalar.activation(tng[:], ps[:, 3 * RT4 : RT, :], AF.Tanh)
                ig = ew.tile([128, RT4, Bs], dt.float32, tag="ig")
                fc = ew.tile([128, RT4, Bs], dt.float32, tag="fc")
                nc.vector.tensor_mul(ig[:], sig[:, 0:RT4, :], tng[:])
                nc.vector.tensor_mul(fc[:], sig[:, RT4 : 2 * RT4, :], c_sb[:])
                nc.vector.tensor_add(c_sb[:], ig[:], fc[:])
                tcn = ew.tile([128, RT4, Bs], dt.float32, tag="tc")
                nc.scalar.activation(tcn[:], c_sb[:], AF.Tanh)
                if use_remote:
                    hdst = hT[:, nxt, 0:RT4, :]
                else:
                    hdst = hT[:, nxt, :, :]
                nc.vector.tensor_mul(hdst, sig[:, 2 * RT4 : 3 * RT4, :], tcn[:])

                if use_remote:
                    with tc.tile_critical():
                        for d in (1, 2, 3):
                            nc.gpsimd.remote_dma_broadcast(
                                hT[:, nxt, d, :],
                                hT[:, nxt, 0, :],
                                remote_sem=h_sem,
                                local_sem=send_sem,
                                rdests=[(0, d)] + [None] * 7,
                            )
                        nc.gpsimd.trigger_dma(count=None)

                if rep == 0:
                    nc.sync.dma_start(ysT_d[t], hT[:, nxt, 0:RT4, :])

                if t == Tn - 1 and rep == 0:
                    hf = ew.tile([128, RT4, Bs], dt.float32, tag="hf")
                    nc.vector.tensor_mul(hf[:], sig[:, 2 * RT4 : 3 * RT4, :], tcn[:])
                    nc.sync.dma_start(hfin_d[:], hf[:])
                    nc.sync.dma_start(cfin_d[:], c_sb[:])

                if rep == 0:
                    emit_interleaved_gemm(t)

    nc.compile()
    return nc


def _rows_for(mode, rank):
    """Global gate-row indices (into 4H) for this core, in row-tile order."""
    cfg = _cfg(mode)
    RT = cfg["RT"]
    if mode == "local":
        return np.concatenate([512 * q + np.arange(512) for q in GATE_ORDER])
    else:
        return np.concatenate(
            [512 * q + 128 * rank + np.arange(128) for q in GATE_ORDER]
        )


_SEL_DIR_CACHE = {}


def _prep_core_sel(c, x, h0, c0, Wi, Wh, bi, bh):
    Bs = 16
    d, rank = c // 4, c % 4
    bsl = slice(rank * Bs, (rank + 1) * Bs)

    # weights / constants are identical across the 4 cores of a direction
    key = (d, id(Wi), id(Wh))
    if key not in _SEL_DIR_CACHE:
        rows = np.concatenate([512 * q + np.arange(512) for q in GATE_ORDER])
        sel = np.zeros((8, 128, Bs), dtype=BF16)
        for s in range(8):
            for j in range(Bs):
                sel[s, Bs * s + j, j] = 1
        _SEL_DIR_CACHE[key] = {
            "wiT": np.ascontiguousarray(Wi[rows].astype(BF16).T.reshape(4, 128, 2048)),
            "whT": np.ascontiguousarray(Wh[rows].astype(BF16).T.reshape(4, 128, 2048)),
            "bias": np.ascontiguousarray((bi + bh)[rows].astype(BF16).reshape(1, 2048)),
            "ones": np.ones((1, 128), dtype=BF16),
            "sel": sel,
            "id16": np.eye(16, dtype=BF16),
            "xrev": np.ascontiguousarray(x[::-1]).astype(BF16) if d == 1 else x.astype(BF16),
        }
    dc = _SEL_DIR_CACHE[key]

    xx = dc["xrev"][:, bsl, :]
    Tn = xx.shape[0]
    xT = np.ascontiguousarray(xx.transpose(2, 0, 1).reshape(4, 128, Tn * Bs))
    h0T = np.stack([h0[bsl, 128 * j : 128 * j + 128].T.astype(BF16) for j in range(4)], axis=1)
    return {
        "xT": xT,
        "wiT": dc["wiT"],
        "whT": dc["whT"],
        "bias": dc["bias"],
        "ones": dc["ones"],
        "sel": dc["sel"],
        "id16": dc["id16"],
        "h0T": np.ascontiguousarray(h0T),
        "c0": np.ascontiguousarray(c0[bsl].astype(F32)),
    }


def _prep_core(mode, c, x, h0, c0, Wi, Wh, bi, bh):
    if mode == "sel":
        m = _prep_core_sel(c, x, h0, c0, Wi, Wh, bi, bh)
        if SEL_FUSED or not SEL_PACKED:
            return m
        offs, total = _sel_blob_layout(x.shape[0])
        blob = np.empty(total, dtype=BF16)
        m["c0u"] = np.ascontiguousarray(m.pop("c0")).view(np.uint16).view(BF16)
        for name, (o, n, shape) in offs.items():
            blob[o : o + n] = np.ascontiguousarray(m[name]).ravel()
        return {"blob": blob}
    cfg = _cfg(mode)
    Bs, RT = cfg["Bs"], cfg["RT"]
    RT4 = RT // 4
    d, rank = c // 4, c % 4
    rows = _rows_for(mode, rank)

    if mode == "local":
        bsl = slice(rank * Bs, (rank + 1) * Bs)
        hsl = np.arange(H)
        slot_slices = np.arange(4)  # hT slot j <- H-slice j
    else:
        bsl = slice(0, B)
        hsl = 128 * rank + np.arange(128)
        slot_slices = np.array([rank ^ j for j in range(4)])

    xx = x[::-1] if d == 1 else x
    xx = xx[:, bsl, :]  # [T, Bs, I]
    Tn = xx.shape[0]
    xT = np.ascontiguousarray(
        xx.astype(BF16).transpose(2, 0, 1).reshape(4, 128, Tn * Bs)
    )

    wi = Wi[rows].astype(BF16)  # [RT*128, I]
    wiT = np.ascontiguousarray(wi.T.reshape(4, 128, RT * 128))
    wh = Wh[rows].astype(BF16).T  # [H, RT*128]
    whT = np.stack(
        [wh[128 * s : 128 * s + 128] for s in slot_slices], axis=0
    )  # [4, 128, RT*128]
    bias = (bi + bh)[rows].astype(F32).reshape(RT, 128).T.copy()  # [128, RT]

    h0T = np.stack(
        [h0[bsl, 128 * s : 128 * s + 128].T.astype(BF16) for s in slot_slices], axis=1
    )  # [128, 4, Bs]
    if mode == "local":
        c0T = np.ascontiguousarray(c0[bsl].T.astype(F32).reshape(RT4, 128, Bs).transpose(1, 0, 2))
    else:
        c0T = c0[bsl, hsl.min() : hsl.min() + 128].T.astype(F32).reshape(128, 1, Bs)

    return {
        "xT": xT,
        "wiT": np.ascontiguousarray(wiT),
        "whT": np.ascontiguousarray(whT),
        "bias": np.ascontiguousarray(bias),
        "h0T": np.ascontiguousarray(h0T),
        "c0T": np.ascontiguousarray(c0T),
        "ident": np.eye(128, dtype=BF16),
    }


def kernel(x, h0_f, c0_f, h0_b, c0_b, Wi_f, Wh_f, bi_f, bh_f, Wi_b, Wh_b, bi_b, bh_b):
    mode = MODE
    cfg = _cfg(mode)
    Bs, RT = cfg["Bs"], cfg["RT"]
    RT4 = RT // 4
    x = np.asarray(x, dtype=F32)
    Tn = x.shape[0]
    _SEL_DIR_CACHE.clear()

    if mode not in _GRAPH_CACHE:
        _GRAPH_CACHE[mode] = build_graph(mode, Tn)
    nc = _GRAPH_CACHE[mode]

    per_dir = [
        (h0_f, c0_f, Wi_f, Wh_f, bi_f, bh_f),
        (h0_b, c0_b, Wi_b, Wh_b, bi_b, bh_b),
    ]
    in_maps = []
    for c in range(8):
        h0, c0, Wi, Wh, bi, bh = [np.asarray(a, dtype=F32) for a in per_dir[c // 4]]
        in_maps.append(_prep_core(mode, c, x, h0, c0, Wi, Wh, bi, bh))

    res = bass_utils.run_bass_kernel_spmd(
        nc, in_maps, core_ids=list(range(8)), trace=TRACE
    )
    global LAST_RESULT
    LAST_RESULT = res

    out = np.zeros((Tn, B, 2 * H), dtype=F32)
    hf = np.zeros((B, H), dtype=F32)
    cf = np.zeros((B, H), dtype=F32)
    hb = np.zeros((B, H), dtype=F32)
    cb = np.zeros((B, H), dtype=F32)
    for c in range(8):
        d, rank = c // 4, c % 4
        r = res.results[c]
        if mode == "sel":
            # bf16 -> f32 cast happens once, during the assignment into `out`
            ys = np.asarray(r["ysT"]).transpose(1, 0, 2)  # [T, Bs, H] bf16 view
            hfin = np.asarray(r["hfin"]).astype(F32)
            cfin = np.asarray(r["cfin"]).astype(F32)
        else:
            ys = np.asarray(r["ysT"]).astype(F32).reshape(Tn, 128, RT4, Bs)
            hfin = np.asarray(r["hfin"]).astype(F32).reshape(128, RT4, Bs)
            cfin = np.asarray(r["cfin"]).astype(F32).reshape(128, RT4, Bs)
            ys = ys.transpose(0, 3, 2, 1).reshape(Tn, Bs, RT4 * 128)  # [T, Bs, dims]
            hfin = hfin.transpose(2, 1, 0).reshape(Bs, RT4 * 128)
            cfin = cfin.transpose(2, 1, 0).reshape(Bs, RT4 * 128)
        if d == 1:
            ys = ys[::-1]
        if mode in ("local", "sel"):
            bsl = slice(rank * Bs, (rank + 1) * Bs)
            dsl = slice(0, H)
        else:
            bsl = slice(0, B)
            dsl = slice(rank * 128, rank * 128 + 128)
        out[:, bsl, (d * H + dsl.start) : (d * H + dsl.stop)] = ys
        (hf if d == 0 else hb)[bsl, dsl] = hfin
        (cf if d == 0 else cb)[bsl, dsl] = cfin

    return out, hf, cf, hb, cb


if __name__ == "__main__":
    rng = np.random.default_rng(0)
    ins = {
        "x": rng.standard_normal((T, B, I), dtype=F32),
        "h0_f": np.zeros((B, H), F32),
        "c0_f": np.zeros((B, H), F32),
        "h0_b": np.zeros((B, H), F32),
        "c0_b": np.zeros((B, H), F32),
    }
    for dd in ("f", "b"):
        ins[f"Wi_{dd}"] = (rng.standard_normal((4 * H, I), dtype=F32) / np.sqrt(I)).astype(F32)
        ins[f"Wh_{dd}"] = (rng.standard_normal((4 * H, H), dtype=F32) / np.sqrt(H)).astype(F32)
        ins[f"bi_{dd}"] = rng.standard_normal(4 * H, dtype=F32) / np.sqrt(H)
        ins[f"bh_{dd}"] = rng.standard_normal(4 * H, dtype=F32) / np.sqrt(H)
    outs = kernel(**ins)
    print([o.shape for o in outs])


# revision 43
# speedup vs baseline: 1.0288x; 1.0288x over previous
"""Bidirectional LSTM layer (T=256, B=64, I=H=512) on 8 TRN2 NeuronCores.

Sharding (all modes): core = direction(2) x batch-shard(4), 16 samples per
core. The backward direction is handled by feeding time-reversed x to cores
4-7 and re-reversing on the host, so all 8 cores run one identical SPMD
graph. All shard/unshard work is numpy inside kernel(). Numerics: bf16
weights/activations into the PE, f32 PSUM accumulation and f32 cell state
(validated 1.1e-2 absmax rel err vs the f32 reference).

MODE="sel" (default, instruction-minimal -- execution on this rig is priced
per instruction, not per FLOP): gates live in [batch, gate] orientation.
  - xg(+biases) for 8 timesteps at a time is one [128(=8 steps x 16 batch),
    2048] accumulation group (x-chunks stationary, WiT streamed; biases
    injected by a K=1 ones-row outer-product matmul).
  - per step: a selector-matrix matmul pulls that step's 16 xg rows into the
    gate PSUM [16, 2048], then 16 matmuls (h-chunks stationary, WhT streamed)
    add the recurrent term; sigmoid/tanh + cell update on [16, 512*k] tiles;
    one dma_start_transpose returns h to the [128, batch] stationary layout.

MODE="local": orientation-B variant (Wh tiles stationary, hT streamed, xg
pre-GEMM resident in SBUF, identity-matmul xg injection). ~2.3x more
instructions; better under cost models that charge per streamed column.

MODE="tp4": gate-tensor-parallel across 4-core groups with per-step h
exchange via remote_dma_broadcast -- unusable on this rig (remote DMA
crashes the runtime) but kept for environments where it works.
"""

import sys

for p in ("/opt/trn_rl_repo",):
    if p not in sys.path:
        sys.path.insert(0, p)

import numpy as np
import ml_dtypes

import concourse.bass as bass
import concourse.tile as tile
import concourse.mybir as mybir
from concourse import bacc, bass_utils

BF16 = ml_dtypes.bfloat16
F32 = np.float32

T, B, I, H = 256, 64, 512, 512
GATE_ORDER = [0, 1, 3, 2]  # pytorch i,f,g,o -> our row-tile order i,f,o,g

MODE = "sel"  # "sel" (instruction-minimal), "local", or "tp4" (needs remote_dma)
# Fused variant (recurrence accumulating straight into the GEMM PSUM block)
# passes CoreSim but fails walrus BIR verification: engine accesses need
# 32-aligned partition bases, and odd steps would read gates at offset 16.
# The selector matmul in the default path is the legal 16-row extraction.
SEL_FUSED = False
# Packing the 9 input arrays into one bf16 blob measured as a wash (A/B within
# +-0.15s noise): the transfer floor is bandwidth-bound, not per-array.
SEL_PACKED = False

TRACE = False  # set by test harness to capture neuron-profile timing
LAST_RESULT = None  # BassKernelResults of the most recent run

_GRAPH_CACHE = {}


def _cfg(mode):
    if mode == "local":
        # Bs batch per core, RT gate row-tiles per core (16 = all 2048 rows)
        return dict(Bs=16, RT=16, use_remote=False)
    elif mode == "tp4":
        return dict(Bs=64, RT=4, use_remote=True)
    elif mode == "sel":
        return dict(Bs=16, RT=16, use_remote=False)
    raise ValueError(mode)


def build_graph_sel_fused(Tn=T):
    """sel-mode refinement: no selector matmuls, no xg staging.

    The recurrence W-matmuls accumulate directly into the 8-step GEMM PSUM
    block. PE matmul outputs must start at a 32-aligned partition, so the
    stationary h operand is padded to 32 columns with a zero half selected by
    step parity: the real result lands on the step's 16 rows, zeros
    accumulate (+0) onto the neighbor step's rows. Per step: 16 matmul pairs
    + activations + one DMA transpose.
    """
    Bs, G = 16, 2048
    dt = mybir.dt
    NBLK = Tn // 8

    nc = bacc.Bacc("TRN2", target_bir_lowering=False, debug=False, num_devices=8)

    xT_d = nc.dram_tensor("xT", [4, 128, Tn * Bs], dt.bfloat16, kind="ExternalInput").ap()
    wiT_d = nc.dram_tensor("wiT", [4, 128, G], dt.bfloat16, kind="ExternalInput").ap()
    whT_d = nc.dram_tensor("whT", [4, 128, G], dt.bfloat16, kind="ExternalInput").ap()
    bias_d = nc.dram_tensor("bias", [1, G], dt.bfloat16, kind="ExternalInput").ap()
    ones_d = nc.dram_tensor("ones", [1, 128], dt.bfloat16, kind="ExternalInput").ap()
    sel_d = nc.dram_tensor("sel", [8, 128, Bs], dt.bfloat16, kind="ExternalInput").ap()
    h0T_d = nc.dram_tensor("h0T", [128, 4, Bs], dt.bfloat16, kind="ExternalInput").ap()
    c0_d = nc.dram_tensor("c0", [Bs, 512], dt.float32, kind="ExternalInput").ap()
    id16_d = nc.dram_tensor("id16", [16, 16], dt.bfloat16, kind="ExternalInput").ap()

    ysT_d = nc.dram_tensor("ysT", [Bs, Tn, 512], dt.bfloat16, kind="ExternalOutput").ap()
    hfin_d = nc.dram_tensor("hfin", [Bs, 512], dt.float32, kind="ExternalOutput").ap()
    cfin_d = nc.dram_tensor("cfin", [Bs, 512], dt.float32, kind="ExternalOutput").ap()

    AF = mybir.ActivationFunctionType

    with tile.TileContext(nc) as tc:
        with (
            tc.tile_pool(name="persist", bufs=1) as persist,
            tc.tile_pool(name="gps", bufs=2, space="PSUM") as gpsum,
            tc.tile_pool(name="ew", bufs=2) as ew,
            tc.tile_pool(name="hsb", bufs=2) as hsbp,
        ):
            xT = persist.tile([128, 4, Tn * Bs], dt.bfloat16, tag="xT")
            wiT = persist.tile([128, 4, G], dt.bfloat16, tag="wiT")
            whT = persist.tile([128, 4, G], dt.bfloat16, tag="whT")
            bias = persist.tile([1, G], dt.bfloat16, tag="bias")
            ones = persist.tile([1, 128], dt.bfloat16, tag="ones")
            # ring slot r holds h in column half r, zeros in the other half
            hT = persist.tile([128, 2, 4, 2 * Bs], dt.bfloat16, tag="hT")
            c_sb = persist.tile([Bs, 512], dt.float32, tag="c_sb")

            for k in range(4):
                nc.sync.dma_start(xT[:, k, :], xT_d[k])
                nc.sync.dma_start(wiT[:, k, :], wiT_d[k])
                nc.sync.dma_start(whT[:, k, :], whT_d[k])
            nc.sync.dma_start(bias[:], bias_d[:])
            nc.sync.dma_start(ones[:], ones_d[:])
            nc.gpsimd.memset(hT[:], 0.0)
            nc.sync.dma_start(hT[:, 0, :, 0:Bs], h0T_d[:])
            nc.sync.dma_start(c_sb[:], c0_d[:])

            ps_tiles = {}

            def emit_gemm_block(blk):
                ps = gpsum.tile([128, G], dt.float32, tag="gps", name=f"gps{blk}")
                for n in range(4):
                    nsl = slice(n * 512, (n + 1) * 512)
                    nc.tensor.matmul(
                        ps[:, nsl], ones[:], bias[:, nsl],
                        start=True, stop=False, skip_group_check=True,
                    )
                for k in range(4):
                    xsl = xT[:, k, blk * 128 : (blk + 1) * 128]
                    for n in range(4):
                        nsl = slice(n * 512, (n + 1) * 512)
                        nc.tensor.matmul(
                            ps[:, nsl], xsl, wiT[:, k, nsl],
                            start=False, stop=False, skip_group_check=True,
                        )
                ps_tiles[blk] = ps

            emit_gemm_block(0)

            for t in range(Tn):
                blk, ph = t // 8, t % 8
                cur, nxt = t % 2, (t + 1) % 2
                if ph == 4 and blk + 1 < NBLK:
                    emit_gemm_block(blk + 1)
                ps = ps_tiles[blk]
                off = 32 * (ph // 2)
                rsl = slice(16 * ph, 16 * ph + 16)

                for k in range(4):
                    hsl = hT[:, cur, k, :]
                    for n in range(4):
                        nsl = slice(n * 512, (n + 1) * 512)
                        nc.tensor.matmul(
                            ps[off : off + 32, nsl], hsl, whT[:, k, nsl],
                            start=False, stop=(ph == 7 and k == 3),
                            skip_group_check=True, tile_position=(0, off),
                        )

                # gate columns: [i | f | o | g]; this step's rows = rsl
                sig = ew.tile([Bs, 1536], dt.float32, tag="sig")
                tng = ew.tile([Bs, 512], dt.float32, tag="tng")
                nc.scalar.activation(sig[:], ps[rsl, 0:1536], AF.Sigmoid)
                nc.scalar.activation(tng[:], ps[rsl, 1536:2048], AF.Tanh)
                ig = ew.tile([Bs, 512], dt.float32, tag="ig")
                fc = ew.tile([Bs, 512], dt.float32, tag="fc")
                nc.vector.tensor_mul(ig[:], sig[:, 0:512], tng[:])
                nc.vector.tensor_mul(fc[:], sig[:, 512:1024], c_sb[:])
                nc.vector.tensor_add(c_sb[:], ig[:], fc[:])
                tcn = ew.tile([Bs, 512], dt.float32, tag="tc")
                nc.scalar.activation(tcn[:], c_sb[:], AF.Tanh)

                if ph == 0:
                    h_sb = hsbp.tile([Bs, 8, 512], dt.bfloat16, tag="h_sb", name=f"hsb{blk}")
                nc.vector.tensor_mul(h_sb[:, ph, :], sig[:, 1024:1536], tcn[:])

                if t < Tn - 1:
                    nc.sync.dma_start_transpose(
                        hT[:, nxt, :, 16 * nxt : 16 * nxt + 16], h_sb[:, ph, :]
                    )

                if ph == 7:
                    nc.sync.dma_start(ysT_d[:, t - 7 : t + 1, :], h_sb[:, :, :])
                if t == Tn - 1:
                    hf = ew.tile([Bs, 512], dt.float32, tag="hf")
                    nc.vector.tensor_mul(hf[:], sig[:, 1024:1536], tcn[:])
                    nc.sync.dma_start(hfin_d[:], hf[:])
                    nc.sync.dma_start(cfin_d[:], c_sb[:])

    _dedup_ldweights(nc)
    nc.compile()
    return nc


def _sel_blob_layout(Tn):
    """(name, elems, shape) of each bf16 segment in the packed input blob.
    c0 (f32) rides as raw bytes viewed as 2x bf16 elements."""
    Bs, G = 16, 2048
    TBs = Tn * Bs
    segs = [
        ("xT", 4 * 128 * TBs, (4, 128, TBs)),
        ("wiT", 4 * 128 * G, (4, 128, G)),
        ("whT", 4 * 128 * G, (4, 128, G)),
        ("bias", G, (1, G)),
        ("ones", 128, (1, 128)),
        ("sel", 8 * 128 * Bs, (8, 128, Bs)),
        ("h0T", 128 * 4 * Bs, (128, 4, Bs)),
        ("c0u", Bs * 1024, (Bs, 1024)),
    ]
    offs, o = {}, 0
    for name, n, shape in segs:
        offs[name] = (o, n, shape)
        o += n
    return offs, o


def build_graph_sel_packed(Tn=T):
    """sel graph with all inputs packed into one bf16 blob (the axon tunnel
    charges ~20ms per array per call, so 9 arrays -> 1 saves ~0.15-0.2s)."""
    Bs, G = 16, 2048
    dt = mybir.dt
    NBLK = Tn // 8
    offs, total = _sel_blob_layout(Tn)

    nc = bacc.Bacc("TRN2", target_bir_lowering=False, debug=False, num_devices=8)
    blob_d = nc.dram_tensor("blob", [total], dt.bfloat16, kind="ExternalInput").ap()
    ysT_d = nc.dram_tensor("ysT", [Bs, Tn, 512], dt.bfloat16, kind="ExternalOutput").ap()
    hfin_d = nc.dram_tensor("hfin", [Bs, 512], dt.float32, kind="ExternalOutput").ap()
    cfin_d = nc.dram_tensor("cfin", [Bs, 512], dt.float32, kind="ExternalOutput").ap()

    def seg(name, idx=None):
        o, n, shape = offs[name]
        if idx is not None:  # slice leading dim
            per = n // shape[0]
            sl = blob_d[o + idx * per : o + (idx + 1) * per]
            rest = shape[1:]
            if len(rest) == 1:
                return sl.rearrange("(a) -> a")
            return sl.rearrange("(p f) -> p f", p=rest[0])
        sl = blob_d[o : o + n]
        if len(shape) == 2:
            return sl.rearrange("(p f) -> p f", p=shape[0])
        return sl.rearrange("(p k f) -> p k f", p=shape[0], k=shape[1])

    AF = mybir.ActivationFunctionType
    with tile.TileContext(nc) as tc:
        with (
            tc.tile_pool(name="persist", bufs=1) as persist,
            tc.tile_pool(name="xgp", bufs=2) as xgp,
            tc.tile_pool(name="gps", bufs=1, space="PSUM") as gpsum,
            tc.tile_pool(name="rps", bufs=1, space="PSUM") as rpsum,
            tc.tile_pool(name="ew", bufs=4) as ew,
            tc.tile_pool(name="hsb", bufs=2) as hsbp,
        ):
            xT = persist.tile([128, 4, Tn * Bs], dt.bfloat16, tag="xT")
            wiT = persist.tile([128, 4, G], dt.bfloat16, tag="wiT")
            whT = persist.tile([128, 4, G], dt.bfloat16, tag="whT")
            bias = persist.tile([1, G], dt.bfloat16, tag="bias")
            ones = persist.tile([1, 128], dt.bfloat16, tag="ones")
            sel = persist.tile([128, 8, Bs], dt.bfloat16, tag="sel")
            hT = persist.tile([128, 2, 4, Bs], dt.bfloat16, tag="hT")
            c_sb = persist.tile([Bs, 512], dt.float32, tag="c_sb")

            for k in range(4):
                nc.sync.dma_start(xT[:, k, :], seg("xT", k))
                nc.sync.dma_start(wiT[:, k, :], seg("wiT", k))
                nc.sync.dma_start(whT[:, k, :], seg("whT", k))
            nc.sync.dma_start(bias[:], seg("bias"))
            nc.sync.dma_start(ones[:], seg("ones"))
            for s in range(8):
                nc.sync.dma_start(sel[:, s, :], seg("sel", s))
            nc.sync.dma_start(hT[:, 0, :, :], seg("h0T"))
            nc.sync.dma_start(c_sb[:].bitcast(dt.bfloat16), seg("c0u"))

            xg_tiles = {}

            def emit_gemm_block(blk):
                ps = gpsum.tile([128, G], dt.float32, tag="gps", name=f"gps{blk}")
                for n in range(4):
                    nsl = slice(n * 512, (n + 1) * 512)
                    nc.tensor.matmul(
                        ps[:, nsl], ones[:], bias[:, nsl],
                        start=True, stop=False, skip_group_check=True,
                    )
                for k in range(4):
                    xsl = xT[:, k, blk * 128 : (blk + 1) * 128]
                    for n in range(4):
                        nsl = slice(n * 512, (n + 1) * 512)
                        nc.tensor.matmul(
                            ps[:, nsl], xsl, wiT[:, k, nsl],
                            start=False, stop=(k == 3), skip_group_check=True,
                        )
                xg = xgp.tile([128, G], dt.bfloat16, tag="xg", name=f"xg{blk}")
                nc.scalar.activation(xg[:, 0:1024], ps[:, 0:1024], AF.Copy)
                nc.vector.tensor_copy(xg[:, 1024:2048], ps[:, 1024:2048])
                xg_tiles[blk] = xg

            emit_gemm_block(0)

            for t in range(Tn):
                blk, ph = t // 8, t % 8
                cur, nxt = t % 2, (t + 1) % 2
                if ph == 4 and blk + 1 < NBLK:
                    emit_gemm_block(blk + 1)
                xg = xg_tiles[blk]

                ps = rpsum.tile([Bs, G], dt.float32, tag="rps", name=f"rps{t}")
                for n in range(4):
                    nsl = slice(n * 512, (n + 1) * 512)
                    nc.tensor.matmul(
                        ps[:, nsl], sel[:, ph, :], xg[:, nsl],
                        start=True, stop=False, skip_group_check=True,
                    )
                for k in range(4):
                    hsl = hT[:, cur, k, :]
                    for n in range(4):
                        nsl = slice(n * 512, (n + 1) * 512)
                        nc.tensor.matmul(
                            ps[:, nsl], hsl, whT[:, k, nsl],
                            start=False, stop=(k == 3), skip_group_check=True,
                        )

                sig = ew.tile([Bs, 1536], dt.float32, tag="sig")
                tng = ew.tile([Bs, 512], dt.float32, tag="tng")
                nc.scalar.activation(sig[:], ps[:, 0:1536], AF.Sigmoid)
                nc.scalar.activation(tng[:], ps[:, 1536:2048], AF.Tanh)
                ig = ew.tile([Bs, 512], dt.float32, tag="ig")
                fc = ew.tile([Bs, 512], dt.float32, tag="fc")
                nc.vector.tensor_mul(ig[:], sig[:, 0:512], tng[:])
                nc.vector.tensor_mul(fc[:], sig[:, 512:1024], c_sb[:])
                nc.vector.tensor_add(c_sb[:], ig[:], fc[:])
                tcn = ew.tile([Bs, 512], dt.float32, tag="tc")
                nc.scalar.activation(tcn[:], c_sb[:], AF.Tanh)

                if ph == 0:
                    h_sb = hsbp.tile([Bs, 8, 512], dt.bfloat16, tag="h_sb", name=f"hsb{blk}")
                nc.vector.tensor_mul(h_sb[:, ph, :], sig[:, 1024:1536], tcn[:])

                if t < Tn - 1:
                    nc.sync.dma_start_transpose(hT[:, nxt, :, :], h_sb[:, ph, :])

                if ph == 7:
                    nc.sync.dma_start(ysT_d[:, t - 7 : t + 1, :], h_sb[:, :, :])
                if t == Tn - 1:
                    hf = ew.tile([Bs, 512], dt.float32, tag="hf")
                    nc.vector.tensor_mul(hf[:], sig[:, 1024:1536], tcn[:])
                    nc.sync.dma_start(hfin_d[:], hf[:])
                    nc.sync.dma_start(cfin_d[:], c_sb[:])

    _dedup_ldweights(nc)
    nc.compile()
    return nc


def _dedup_ldweights(nc):
    """Drop InstLdweights that reload the exact weights AP already resident in
    the PE array (no intervening PE instruction other than matmuls). The PE
    retains the stationary operand across matmuls, and standalone-LDW +
    non-self-loading matmul is valid for non-f32 dtypes. Only sync-free LDWs
    are dropped so no semaphore edges are lost. Runs on the traced+scheduled
    stream before Bacc.compile()."""
    removed = 0
    PE = mybir.EngineType.PE
    for blk in nc.main_func.blocks:
        last = None
        keep = []
        for ins in blk.instructions:
            if getattr(ins, "engine", None) == PE:
                nm = type(ins).__name__
                if nm == "InstLdweights":
                    si = ins.sync_info
                    clean = si is None or (not si.on_wait and not si.on_update)
                    key = repr(ins.ins[0])
                    if clean and key == last:
                        removed += 1
                        continue
                    last = key
                elif nm != "InstMatmult":
                    last = None
            keep.append(ins)
        blk.instructions[:] = keep
    return removed


def build_graph_sel(Tn=T, use_dma_transpose=True):
    """Instruction-minimal variant: gates in [batch, gate] orientation.

    xg for 8 steps at a time is one [128, 2048] GEMM block (partitions =
    8 steps x 16 batch, bias via a K=1 ones-row matmul); each step's xg rows
    are pulled into the gate PSUM with a selector-matrix matmul, followed by
    h-chunk-stationary matmuls streaming WhT. h goes back to the [128, b]
    stationary layout with one DMA transpose per step.
    """
    Bs, G = 16, 2048
    dt = mybir.dt
    NBLK = Tn // 8

    nc = bacc.Bacc("TRN2", target_bir_lowering=False, debug=False, num_devices=8)

    xT_d = nc.dram_tensor("xT", [4, 128, Tn * Bs], dt.bfloat16, kind="ExternalInput").ap()
    wiT_d = nc.dram_tensor("wiT", [4, 128, G], dt.bfloat16, kind="ExternalInput").ap()
    whT_d = nc.dram_tensor("whT", [4, 128, G], dt.bfloat16, kind="ExternalInput").ap()
    bias_d = nc.dram_tensor("bias", [1, G], dt.bfloat16, kind="ExternalInput").ap()
    ones_d = nc.dram_tensor("ones", [1, 128], dt.bfloat16, kind="ExternalInput").ap()
    sel_d = nc.dram_tensor("sel", [8, 128, Bs], dt.bfloat16, kind="ExternalInput").ap()
    h0T_d = nc.dram_tensor("h0T", [128, 4, Bs], dt.bfloat16, kind="ExternalInput").ap()
    c0_d = nc.dram_tensor("c0", [Bs, 512], dt.float32, kind="ExternalInput").ap()
    id16_d = nc.dram_tensor("id16", [16, 16], dt.bfloat16, kind="ExternalInput").ap()

    ysT_d = nc.dram_tensor("ysT", [Bs, Tn, 512], dt.bfloat16, kind="ExternalOutput").ap()
    hfin_d = nc.dram_tensor("hfin", [Bs, 512], dt.float32, kind="ExternalOutput").ap()
    cfin_d = nc.dram_tensor("cfin", [Bs, 512], dt.float32, kind="ExternalOutput").ap()

    AF = mybir.ActivationFunctionType

    with tile.TileContext(nc) as tc:
        with (
            tc.tile_pool(name="persist", bufs=1) as persist,
            tc.tile_pool(name="xgp", bufs=2) as xgp,
            tc.tile_pool(name="gps", bufs=1, space="PSUM") as gpsum,
            tc.tile_pool(name="rps", bufs=1, space="PSUM") as rpsum,
            tc.tile_pool(name="ew", bufs=4) as ew,
            tc.tile_pool(name="hsb", bufs=2) as hsbp,
        ):
            xT = persist.tile([128, 4, Tn * Bs], dt.bfloat16, tag="xT")
            wiT = persist.tile([128, 4, G], dt.bfloat16, tag="wiT")
            whT = persist.tile([128, 4, G], dt.bfloat16, tag="whT")
            bias = persist.tile([1, G], dt.bfloat16, tag="bias")
            ones = persist.tile([1, 128], dt.bfloat16, tag="ones")
            sel = persist.tile([128, 8, Bs], dt.bfloat16, tag="sel")
            id16 = persist.tile([16, 16], dt.bfloat16, tag="id16")
            hT = persist.tile([128, 2, 4, Bs], dt.bfloat16, tag="hT")
            c_sb = persist.tile([Bs, 512], dt.float32, tag="c_sb")

            for k in range(4):
                nc.sync.dma_start(xT[:, k, :], xT_d[k])
                nc.sync.dma_start(wiT[:, k, :], wiT_d[k])
                nc.sync.dma_start(whT[:, k, :], whT_d[k])
            nc.sync.dma_start(bias[:], bias_d[:])
            nc.sync.dma_start(ones[:], ones_d[:])
            for s in range(8):
                nc.sync.dma_start(sel[:, s, :], sel_d[s])
            nc.sync.dma_start(id16[:], id16_d[:])
            nc.sync.dma_start(hT[:, 0, :, :], h0T_d[:])
            nc.sync.dma_start(c_sb[:], c0_d[:])

            xg_tiles = {}

            def emit_gemm_block(blk):
                # xg for steps 8*blk..8*blk+7: [128 (8 steps x 16 batch), 2048]
                ps = gpsum.tile([128, G], dt.float32, tag="gps", name=f"gps{blk}")
                for n in range(4):
                    nsl = slice(n * 512, (n + 1) * 512)
                    nc.tensor.matmul(
                        ps[:, nsl], ones[:], bias[:, nsl],
                        start=True, stop=False, skip_group_check=True,
                    )
                for k in range(4):
                    xsl = xT[:, k, blk * 128 : (blk + 1) * 128]
                    for n in range(4):
                        nsl = slice(n * 512, (n + 1) * 512)
                        nc.tensor.matmul(
                            ps[:, nsl], xsl, wiT[:, k, nsl],
                            start=False, stop=(k == 3), skip_group_check=True,
                        )
                xg = xgp.tile([128, G], dt.bfloat16, tag="xg", name=f"xg{blk}")
                nc.scalar.activation(xg[:, 0:1024], ps[:, 0:1024], AF.Copy)
                nc.vector.tensor_copy(xg[:, 1024:2048], ps[:, 1024:2048])
                xg_tiles[blk] = xg

            emit_gemm_block(0)

            for t in range(Tn):
                blk, ph = t // 8, t % 8
                cur, nxt = t % 2, (t + 1) % 2
                if ph == 4 and blk + 1 < NBLK:
                    emit_gemm_block(blk + 1)
                xg = xg_tiles[blk]

                ps = rpsum.tile([Bs, G], dt.float32, tag="rps", name=f"rps{t}")
                for n in range(4):
                    nsl = slice(n * 512, (n + 1) * 512)
                    nc.tensor.matmul(
                        ps[:, nsl], sel[:, ph, :], xg[:, nsl],
                        start=True, stop=False, skip_group_check=True,
                    )
                for k in range(4):
                    hsl = hT[:, cur, k, :]
                    for n in range(4):
                        nsl = slice(n * 512, (n + 1) * 512)
                        nc.tensor.matmul(
                            ps[:, nsl], hsl, whT[:, k, nsl],
                            start=False, stop=(k == 3), skip_group_check=True,
                        )

                # gate columns: [i | f | o | g] along the 2048
                sig = ew.tile([Bs, 1536], dt.float32, tag="sig")
                tng = ew.tile([Bs, 512], dt.float32, tag="tng")
                nc.scalar.activation(sig[:], ps[:, 0:1536], AF.Sigmoid)
                nc.scalar.activation(tng[:], ps[:, 1536:2048], AF.Tanh)
                ig = ew.tile([Bs, 512], dt.float32, tag="ig")
                fc = ew.tile([Bs, 512], dt.float32, tag="fc")
                nc.vector.tensor_mul(ig[:], sig[:, 0:512], tng[:])
                nc.vector.tensor_mul(fc[:], sig[:, 512:1024], c_sb[:])
                nc.vector.tensor_add(c_sb[:], ig[:], fc[:])
                tcn = ew.tile([Bs, 512], dt.float32, tag="tc")
                nc.scalar.activation(tcn[:], c_sb[:], AF.Tanh)

                if ph == 0:
                    h_sb = hsbp.tile([Bs, 8, 512], dt.bfloat16, tag="h_sb", name=f"hsb{blk}")
                    xg_tiles[blk - 1] = None  # allow pool reuse
                nc.vector.tensor_mul(h_sb[:, ph, :], sig[:, 1024:1536], tcn[:])

                if t < Tn - 1:
                    if use_dma_transpose:
                        nc.sync.dma_start_transpose(hT[:, nxt, :, :], h_sb[:, ph, :])
                    else:
                        for k in range(4):
                            tp = rpsum.tile([128, Bs], dt.bfloat16, tag="tp", name=f"tp{t}_{k}")
                            nc.tensor.transpose(
                                tp[:], h_sb[:, ph, k * 128 : (k + 1) * 128], id16[:]
                            )
                            nc.vector.tensor_copy(hT[:, nxt, k, :], tp[:])

                if ph == 7:
                    nc.sync.dma_start(ysT_d[:, t - 7 : t + 1, :], h_sb[:, :, :])
                if t == Tn - 1:
                    hf = ew.tile([Bs, 512], dt.float32, tag="hf")
                    nc.vector.tensor_mul(hf[:], sig[:, 1024:1536], tcn[:])
                    nc.sync.dma_start(hfin_d[:], hf[:])
                    nc.sync.dma_start(cfin_d[:], c_sb[:])

    _dedup_ldweights(nc)
    nc.compile()
    return nc


def build_graph(mode, Tn=T, reps=1):
    # reps>1 re-runs the recurrence compute without I/O (timing calibration
    # only -- outputs stay those of rep 0).
    if mode == "sel":
        assert reps == 1
        if SEL_FUSED:
            return build_graph_sel_fused(Tn)
        return build_graph_sel_packed(Tn) if SEL_PACKED else build_graph_sel(Tn)
    cfg = _cfg(mode)
    Bs, RT, use_remote = cfg["Bs"], cfg["RT"], cfg["use_remote"]
    RT4 = RT // 4
    TBs = Tn * Bs
    NCH = 8  # xg chunks along time
    CH = TBs // NCH  # columns per xg chunk
    NPW = min(512, CH)  # N-pass width
    NB = CH // NPW  # N passes per chunk
    dt = mybir.dt

    nc = bacc.Bacc("TRN2", target_bir_lowering=False, debug=False, num_devices=8)

    xT_d = nc.dram_tensor("xT", [4, 128, TBs], dt.bfloat16, kind="ExternalInput").ap()
    wiT_d = nc.dram_tensor("wiT", [4, 128, RT * 128], dt.bfloat16, kind="ExternalInput").ap()
    whT_d = nc.dram_tensor("whT", [4, 128, RT * 128], dt.bfloat16, kind="ExternalInput").ap()
    bias_d = nc.dram_tensor("bias", [128, RT], dt.float32, kind="ExternalInput").ap()
    h0T_d = nc.dram_tensor("h0T", [128, 4, Bs], dt.bfloat16, kind="ExternalInput").ap()
    c0T_d = nc.dram_tensor("c0T", [128, RT4, Bs], dt.float32, kind="ExternalInput").ap()
    id_d = nc.dram_tensor("ident", [128, 128], dt.bfloat16, kind="ExternalInput").ap()

    ysT_d = nc.dram_tensor("ysT", [Tn, 128, RT4 * Bs], dt.bfloat16, kind="ExternalOutput").ap()
    hfin_d = nc.dram_tensor("hfin", [128, RT4 * Bs], dt.float32, kind="ExternalOutput").ap()
    cfin_d = nc.dram_tensor("cfin", [128, RT4 * Bs], dt.float32, kind="ExternalOutput").ap()

    if use_remote:
        h_sem = nc.alloc_semaphore("h_sem")
        send_sem = nc.alloc_semaphore("send_sem")

    AF = mybir.ActivationFunctionType

    with tile.TileContext(nc) as tc:
        with (
            tc.tile_pool(name="persist", bufs=1) as persist,
            tc.tile_pool(name="xc", bufs=3) as xcp,
            tc.tile_pool(name="gpsum", bufs=3, space="PSUM") as gpsum,
            tc.tile_pool(name="spsum", bufs=2, space="PSUM") as spsum,
            tc.tile_pool(name="ew", bufs=2) as ew,
        ):
            wiT = persist.tile([128, 4, RT * 128], dt.bfloat16, tag="wiT")
            whT = persist.tile([128, 4, RT * 128], dt.bfloat16, tag="whT")
            bias = persist.tile([128, RT], dt.float32, tag="bias")
            ident = persist.tile([128, 128], dt.bfloat16, tag="ident")
            hT = persist.tile([128, 2, 4, Bs], dt.bfloat16, tag="hT")
            c_sb = persist.tile([128, RT4, Bs], dt.float32, tag="c_sb")
            xg = [
                persist.tile([128, RT, CH], dt.bfloat16, tag=f"xg{ch}", name=f"xg{ch}")
                for ch in range(NCH)
            ]

            for k in range(4):
                nc.sync.dma_start(wiT[:, k, :], wiT_d[k])
                nc.sync.dma_start(whT[:, k, :], whT_d[k])
            nc.sync.dma_start(bias[:], bias_d[:])
            nc.sync.dma_start(ident[:], id_d[:])
            nc.sync.dma_start(hT[:, 0, :, :], h0T_d[:])
            nc.sync.dma_start(c_sb[:], c0T_d[:])

            # ---- xg = x @ Wi^T + bias, bf16, resident in SBUF ----
            # One "work item" = (ch, nb, rt): 4 matmuls + 1 bias epilogue. The
            # xc chunk DMA is issued at the first item of each (ch, nb).
            xc_tiles = {}

            def emit_gemm_item(ch, nb, rt):
                off = ch * CH + nb * NPW
                if rt == 0:
                    xc = xcp.tile([128, 4, NPW], dt.bfloat16, tag="xc", name=f"xc{ch}_{nb}")
                    for k in range(4):
                        nc.sync.dma_start(xc[:, k, :], xT_d[k, :, off : off + NPW])
                    xc_tiles[(ch, nb)] = xc
                xc = xc_tiles[(ch, nb)]
                ps = gpsum.tile([128, NPW], dt.float32, tag="gps", name=f"gps{ch}_{nb}_{rt}")
                for k in range(4):
                    nc.tensor.matmul(
                        ps[:],
                        wiT[:, k, rt * 128 : (rt + 1) * 128],
                        xc[:, k, :],
                        start=(k == 0),
                        stop=(k == 3),
                    )
                dst = xg[ch][:, rt, nb * NPW : (nb + 1) * NPW]
                if rt % 2 == 0:
                    nc.scalar.activation(
                        dst, ps[:], AF.Identity, bias=bias[:, rt : rt + 1]
                    )
                else:
                    nc.vector.tensor_scalar_add(dst, ps[:], bias[:, rt : rt + 1])

            # chunks 0..PRE-1 upfront; chunks PRE.. are interleaved into the
            # recurrence steps of chunk c-PRE (PE consumes them in its idle
            # windows while the elementwise chain runs).
            PRE = 1
            for ch in range(PRE):
                for nb in range(NB):
                    for rt in range(RT):
                        emit_gemm_item(ch, nb, rt)

            TCH_steps = Tn // NCH
            items_per_step = -(-(NB * RT) // TCH_steps)  # ceil

            def emit_interleaved_gemm(t):
                ch = t // TCH_steps + PRE
                if ch >= NCH:
                    return
                pos = t % TCH_steps
                for it in range(pos * items_per_step, min((pos + 1) * items_per_step, NB * RT)):
                    emit_gemm_item(ch, it // RT, it % RT)

            # ---- recurrence ----
            for rep in range(reps):
              for t in range(Tn):
                cur, nxt = t % 2, (t + 1) % 2
                ch, tloc = t // TCH_steps, t % TCH_steps
                xg_t = xg[ch]
                ps = spsum.tile([128, RT, Bs], dt.float32, tag="sps")

                # xg injection via identity matmuls (N = RT*Bs/n_idmm <= 512)
                n_idmm = max(1, (RT * Bs) // 512)
                rt_per = RT // n_idmm
                for q in range(n_idmm):
                    nc.tensor.matmul(
                        ps[:, q * rt_per : (q + 1) * rt_per, :],
                        ident[:],
                        xg_t[:, q * rt_per : (q + 1) * rt_per, tloc * Bs : (tloc + 1) * Bs],
                        start=(q == 0),
                        stop=False,
                        skip_group_check=True,
                    )

                def wh_mms():
                    for rt in range(RT):
                        for k in range(4):
                            nc.tensor.matmul(
                                ps[:, rt, :],
                                whT[:, k, rt * 128 : (rt + 1) * 128],
                                hT[:, cur, k, :],
                                start=False,
                                stop=(rt == RT - 1 and k == 3),
                                skip_group_check=True,
                            )

                if use_remote and t >= 1:
                    with tc.tile_critical():
                        nc.tensor.wait_ge(h_sem, 6 * t)
                        wh_mms()
                else:
                    wh_mms()

                # elementwise: row-tiles [0:RT4]=i [RT4:2RT4]=f [2RT4:3RT4]=o [3RT4:RT]=g
                sig = ew.tile([128, 3 * RT4, Bs], dt.float32, tag="sig")
                tng = ew.tile([128, RT4, Bs], dt.float32, tag="tng")
                nc.scalar.activation(sig[:], ps[:, 0 : 3 * RT4, :], AF.Sigmoid)
                nc.scalar.activation(tng[:], ps[:, 3 * RT4 : RT, :], AF.Tanh)
                ig = ew.tile([128, RT4, Bs], dt.float32, tag="ig")
                fc = ew.tile([128, RT4, Bs], dt.float32, tag="fc")
                nc.vector.tensor_mul(ig[:], sig[:, 0:RT4, :], tng[:])
                nc.vector.tensor_mul(fc[:], sig[:, RT4 : 2 * RT4, :], c_sb[:])
                nc.vector.tensor_add(c_sb[:], ig[:], fc[:])
                tcn = ew.tile([128, RT4, Bs], dt.float32, tag="tc")
                nc.scalar.activation(tcn[:], c_sb[:], AF.Tanh)
                if use_remote:
                    hdst = hT[:, nxt, 0:RT4, :]
                else:
                    hdst = hT[:, nxt, :, :]
                nc.vector.tensor_mul(hdst, sig[:, 2 * RT4 : 3 * RT4, :], tcn[:])

                if use_remote:
                    with tc.tile_critical():
                        for d in (1, 2, 3):
                            nc.gpsimd.remote_dma_broadcast(
                                hT[:, nxt, d, :],
                                hT[:, nxt, 0, :],
                                remote_sem=h_sem,
                                local_sem=send_sem,
                                rdests=[(0, d)] + [None] * 7,
                            )
                        nc.gpsimd.trigger_dma(count=None)

                if rep == 0:
                    nc.sync.dma_start(ysT_d[t], hT[:, nxt, 0:RT4, :])

                if t == Tn - 1 and rep == 0:
                    hf = ew.tile([128, RT4, Bs], dt.float32, tag="hf")
                    nc.vector.tensor_mul(hf[:], sig[:, 2 * RT4 : 3 * RT4, :], tcn[:])
                    nc.sync.dma_start(hfin_d[:], hf[:])
                    nc.sync.dma_start(cfin_d[:], c_sb[:])

                if rep == 0:
                    emit_interleaved_gemm(t)

    _dedup_ldweights(nc)
    nc.compile()
    return nc


def _rows_for(mode, rank):
    """Global gate-row indices (into 4H) for this core, in row-tile order."""
    cfg = _cfg(mode)
    RT = cfg["RT"]
    if mode == "local":
        return np.concatenate([512 * q + np.arange(512) for q in GATE_ORDER])
    else:
        return np.concatenate(
            [512 * q + 128 * rank + np.arange(128) for q in GATE_ORDER]
        )


_SEL_DIR_CACHE = {}


def _prep_core_sel(c, x, h0, c0, Wi, Wh, bi, bh):
    Bs = 16
    d, rank = c // 4, c % 4
    bsl = slice(rank * Bs, (rank + 1) * Bs)

    # weights / constants are identical across the 4 cores of a direction
    key = (d, id(Wi), id(Wh))
    if key not in _SEL_DIR_CACHE:
        rows = np.concatenate([512 * q + np.arange(512) for q in GATE_ORDER])
        sel = np.zeros((8, 128, Bs), dtype=BF16)
        for s in range(8):
            for j in range(Bs):
                sel[s, Bs * s + j, j] = 1
        _SEL_DIR_CACHE[key] = {
            "wiT": np.ascontiguousarray(Wi[rows].astype(BF16).T.reshape(4, 128, 2048)),
            "whT": np.ascontiguousarray(Wh[rows].astype(BF16).T.reshape(4, 128, 2048)),
            "bias": np.ascontiguousarray((bi + bh)[rows].astype(BF16).reshape(1, 2048)),
            "ones": np.ones((1, 128), dtype=BF16),
            "sel": sel,
            "id16": np.eye(16, dtype=BF16),
            "xrev": np.ascontiguousarray(x[::-1]).astype(BF16) if d == 1 else x.astype(BF16),
        }
    dc = _SEL_DIR_CACHE[key]

    xx = dc["xrev"][:, bsl, :]
    Tn = xx.shape[0]
    xT = np.ascontiguousarray(xx.transpose(2, 0, 1).reshape(4, 128, Tn * Bs))
    h0T = np.stack([h0[bsl, 128 * j : 128 * j + 128].T.astype(BF16) for j in range(4)], axis=1)
    return {
        "xT": xT,
        "wiT": dc["wiT"],
        "whT": dc["whT"],
        "bias": dc["bias"],
        "ones": dc["ones"],
        "sel": dc["sel"],
        "id16": dc["id16"],
        "h0T": np.ascontiguousarray(h0T),
        "c0": np.ascontiguousarray(c0[bsl].astype(F32)),
    }


def _prep_core(mode, c, x, h0, c0, Wi, Wh, bi, bh):
    if mode == "sel":
        m = _prep_core_sel(c, x, h0, c0, Wi, Wh, bi, bh)
        if SEL_FUSED or not SEL_PACKED:
            return m
        offs, total = _sel_blob_layout(x.shape[0])
        blob = np.empty(total, dtype=BF16)
        m["c0u"] = np.ascontiguousarray(m.pop("c0")).view(np.uint16).view(BF16)
        for name, (o, n, shape) in offs.items():
            blob[o : o + n] = np.ascontiguousarray(m[name]).ravel()
        return {"blob": blob}
    cfg = _cfg(mode)
    Bs, RT = cfg["Bs"], cfg["RT"]
    RT4 = RT // 4
    d, rank = c // 4, c % 4
    rows = _rows_for(mode, rank)

    if mode == "local":
        bsl = slice(rank * Bs, (rank + 1) * Bs)
        hsl = np.arange(H)
        slot_slices = np.arange(4)  # hT slot j <- H-slice j
    else:
        bsl = slice(0, B)
        hsl = 128 * rank + np.arange(128)
        slot_slices = np.array([rank ^ j for j in range(4)])

    xx = x[::-1] if d == 1 else x
    xx = xx[:, bsl, :]  # [T, Bs, I]
    Tn = xx.shape[0]
    xT = np.ascontiguousarray(
        xx.astype(BF16).transpose(2, 0, 1).reshape(4, 128, Tn * Bs)
    )

    wi = Wi[rows].astype(BF16)  # [RT*128, I]
    wiT = np.ascontiguousarray(wi.T.reshape(4, 128, RT * 128))
    wh = Wh[rows].astype(BF16).T  # [H, RT*128]
    whT = np.stack(
        [wh[128 * s : 128 * s + 128] for s in slot_slices], axis=0
    )  # [4, 128, RT*128]
    bias = (bi + bh)[rows].astype(F32).reshape(RT, 128).T.copy()  # [128, RT]

    h0T = np.stack(
        [h0[bsl, 128 * s : 128 * s + 128].T.astype(BF16) for s in slot_slices], axis=1
    )  # [128, 4, Bs]
    if mode == "local":
        c0T = np.ascontiguousarray(c0[bsl].T.astype(F32).reshape(RT4, 128, Bs).transpose(1, 0, 2))
    else:
        c0T = c0[bsl, hsl.min() : hsl.min() + 128].T.astype(F32).reshape(128, 1, Bs)

    return {
        "xT": xT,
        "wiT": np.ascontiguousarray(wiT),
        "whT": np.ascontiguousarray(whT),
        "bias": np.ascontiguousarray(bias),
        "h0T": np.ascontiguousarray(h0T),
        "c0T": np.ascontiguousarray(c0T),
        "ident": np.eye(128, dtype=BF16),
    }


def kernel(x, h0_f, c0_f, h0_b, c0_b, Wi_f, Wh_f, bi_f, bh_f, Wi_b, Wh_b, bi_b, bh_b):
    mode = MODE
    cfg = _cfg(mode)
    Bs, RT = cfg["Bs"], cfg["RT"]
    RT4 = RT // 4
    x = np.asarray(x, dtype=F32)
    Tn = x.shape[0]
    _SEL_DIR_CACHE.clear()

    if mode not in _GRAPH_CACHE:
        _GRAPH_CACHE[mode] = build_graph(mode, Tn)
    nc = _GRAPH_CACHE[mode]

    per_dir = [
        (h0_f, c0_f, Wi_f, Wh_f, bi_f, bh_f),
        (h0_b, c0_b, Wi_b, Wh_b, bi_b, bh_b),
    ]
    in_maps = []
    for c in range(8):
        h0, c0, Wi, Wh, bi, bh = [np.asarray(a, dtype=F32) for a in per_dir[c // 4]]
        in_maps.append(_prep_core(mode, c, x, h0, c0, Wi, Wh, bi, bh))

    res = bass_utils.run_bass_kernel_spmd(
        nc, in_maps, core_ids=list(range(8)), trace=TRACE
    )
    global LAST_RESULT
    LAST_RESULT = res

    out = np.zeros((Tn, B, 2 * H), dtype=F32)
    hf = np.zeros((B, H), dtype=F32)
    cf = np.zeros((B, H), dtype=F32)
    hb = np.zeros((B, H), dtype=F32)
    cb = np.zeros((B, H), dtype=F32)
    for c in range(8):
        d, rank = c // 4, c % 4
        r = res.results[c]
        if mode == "sel":
            # bf16 -> f32 cast happens once, during the assignment into `out`
            ys = np.asarray(r["ysT"]).transpose(1, 0, 2)  # [T, Bs, H] bf16 view
            hfin = np.asarray(r["hfin"]).astype(F32)
            cfin = np.asarray(r["cfin"]).astype(F32)
        else:
            ys = np.asarray(r["ysT"]).astype(F32).reshape(Tn, 128, RT4, Bs)
            hfin = np.asarray(r["hfin"]).astype(F32).reshape(128, RT4, Bs)
            cfin = np.asarray(r["cfin"]).astype(F32).reshape(128, RT4, Bs)
            ys = ys.transpose(0, 3, 2, 1).reshape(Tn, Bs, RT4 * 128)  # [T, Bs, dims]
            hfin = hfin.transpose(2, 1, 0).reshape(Bs, RT4 * 128)
            cfin = cfin.transpose(2, 1, 0).reshape(Bs, RT4 * 128)
        if d == 1:
            ys = ys[::-1]
        if mode in ("local", "sel"):
            bsl = slice(rank * Bs, (rank + 1) * Bs)
            dsl = slice(0, H)
        else:
            bsl = slice(0, B)
            dsl = slice(rank * 128, rank * 128 + 128)
        out[:, bsl, (d * H + dsl.start) : (d * H + dsl.stop)] = ys
        (hf if d == 0 else hb)[bsl, dsl] = hfin
        (cf if d == 0 else cb)[bsl, dsl] = cfin

    return out, hf, cf, hb, cb


if __name__ == "__main__":
    rng = np.random.default_rng(0)
    ins = {
        "x": rng.standard_normal((T, B, I), dtype=F32),
        "h0_f": np.zeros((B, H), F32),
        "c0_f": np.zeros((B, H), F32),
        "h0_b": np.zeros((B, H), F32),
        "c0_b": np.zeros((B, H), F32),
    }
    for dd in ("f", "b"):
        ins[f"Wi_{dd}"] = (rng.standard_normal((4 * H, I), dtype=F32) / np.sqrt(I)).astype(F32)
        ins[f"Wh_{dd}"] = (rng.standard_normal((4 * H, H), dtype=F32) / np.sqrt(H)).astype(F32)
        ins[f"bi_{dd}"] = rng.standard_normal(4 * H, dtype=F32) / np.sqrt(H)
        ins[f"bh_{dd}"] = rng.standard_normal(4 * H, dtype=F32) / np.sqrt(H)
    outs = kernel(**ins)
    print([o.shape for o in outs])
